# revision 57
# baseline (speedup 1.0000x reference)
"""Trainium2 Bass kernel for nn_Attention (dense transformer MHA block).

Reference computation (fp32):
    qkv = x @ w_qkv.T            # [B,N,3C]
    q,k,v per head; scores = q k^T / sqrt(D); attn = softmax(scores)
    o = attn @ v;  y = o @ w_proj.T + b_proj

Sharding over 8 NeuronCores (data-parallel over batch x tensor-parallel over
heads): core c -> (batch b = c//4, head group g = c%4, heads 4g..4g+3).
Each core computes q/k/v for its 4 heads over the full 2048-token sequence,
runs attention locally, and multiplies by its row-slice of w_proj, producing
a PARTIAL output [2048, 1024].  The 4 partials per batch are summed on the
host (numpy) together with the bias — no device collectives.

Perf structure (v7, ~209us vs 223us for v2):
  - score matmuls run in 64x64 array-tiling mode: 4 concurrent tiles
    (2 heads x 2 kv-halves) per 512-wide round, ~2x faster than the
    untiled 64-contraction matmuls.
  - exp is split across engines: head A of each pair uses the ACT table
    exp; head B uses a Schraudolph bit-trick exp on the DVE
    (y_i16 = round(s*A + B) bitcast as bf16 == exp(s*scale) within ~4%).
    The per-head softmax normalization cancels each head's systematic
    scale error.
  - A@V keeps the ones-column trick (V gets a 65th column of ones so the
    same matmul accumulates the softmax denominator) - provably optimal:
    A@V is rhs-stream-bound (E enters the PE at 128 lanes/cycle), so any
    separate denominator matmul would re-stream all exp columns.
  - WEAVE startup: only k0/q0 of the first 512-token block precede the
    unit loop; the other 8 qk projection tiles become PE fillers popped
    through unit 0 (2 slots per group), so attention starts as soon as
    x block 0 + the k0/q0 weight columns land (~13us) instead of after
    the full qk phase (~26us), overlapping the remaining x DMA.
  - per-iteration emission order [scores -> exps -> fillers -> A@V]:
    filler matmuls deepen the score->A@V skew on the in-order PE, and
    filler DVE copies queue behind the exps instead of delaying them.
  - normalize: den row staged [1,512] (DVE), 1/den before broadcast
    ([1,512] DVE reciprocal, GPSIMD broadcast of the reciprocal), muls
    deferred ~2 iterations via a readiness queue.  Custom DVE/GPSIMD ops
    (reciprocal_approx_fast, partition_broadcast) MUST read partition-0
    tiles - feeding them partition-offset-64 APs returns garbage (NaN).
  - proj evacuation: one [128,1024] ACT copy per row-block (spans both
    PSUM banks; ACT per-op overhead ~215ns makes two half copies
    slower); final query tile keeps split ACT/DVE halves for tail
    overlap.  qt2's projection is delay-pushed so ~half of it drains
    after the last A@V, keeping the PE busy (and out of the low-clock
    p-state) while the final normalize chain runs.
  - engine-queue discipline (measured, do not "rebalance" casually):
    moving den copies or qk-tile copies to ACT puts DMA/matmul-gated
    ops in front of the exps on an in-order queue and loses 2-37us.
    tensor_tensor(divide) is rejected by codegen (s3s3d3_tt_valid_op);
    fp8 anywhere fails the 2e-2 gate (scores err ~0.12 absolute).
"""

import numpy as np

B, N, C = 2, 2048, 1024
H, D = 16, 64
NCORES = 8
GROUPS = 4              # head groups (tensor-parallel)
HG = H // GROUPS        # 4 heads per core
CG = HG * D             # 256 channels per core
P = 128
KT = C // P             # 8 contraction subtiles for C=1024
KV_CHUNKS = N // P      # 16 key/value chunks of 128 rows
QT = N // 512           # 4 query tiles of 512
VB = D + 1              # v block width incl. ones column (65)
SCALE = 1.0 / float(np.sqrt(D))
# Schraudolph exp constants (bf16 bit-trick on DVE): for scores s (pre-scale),
# exp(s*SCALE) ~= bitcast_bf16(int16(s*EXP_A + EXP_B)).  The -7.63 centers the
# sawtooth approximation error; the per-head softmax cancels the global scale.
EXP_A = 128.0 * 1.4426950408889634 * SCALE
EXP_B = 127.0 * 128.0 - 7.63

import os
FLAG_SCORES_TILED = os.environ.get("K_SCORES_TILED", "1") == "1"
FLAG_EXP_DVE = os.environ.get("K_EXP_DVE", "1") == "1"
FLAG_MUL_GPSIMD = os.environ.get("K_MUL_GPSIMD", "1") == "1"
# v3 knobs
EXP_SPLIT = int(os.environ.get("K_EXP_SPLIT", "1024"))  # DVE cols of scs[1]
FLAG_EXP_CHUNK = os.environ.get("K_EXP_CHUNK", "0") == "1"  # DVE exp per-chunk
# muls read o_acc straight from PSUM: saves the ACT ou copies but the
# deferred-mul WAR on the 2-deep ps1 pool serializes unit boundaries.
FLAG_MUL_PSUM = os.environ.get("K_MUL_PSUM", "0") == "1"
# tensor_tensor divide: REJECTED by codegen (s3s3d3_tt_valid_op) — fp32
# divide is not a valid DVE tensor_tensor op.  Keep recip+mul.
FLAG_DIV = os.environ.get("K_DIV", "0") == "1"
FLAG_DEN_ACT = os.environ.get("K_DEN_ACT", "0") == "1"  # den copy on ACT
FLAG_PP_MERGE = os.environ.get("K_PP_MERGE", "1") == "1"    # 1-op proj evac
FLAG_NORM_V3 = os.environ.get("K_NORM_V3", "1") == "1"
FLAG_DMA_V3 = os.environ.get("K_DMA_V3", "0") == "1"
# v7: weave k0b1..3/k1/q1 into unit 0 as fillers so attention starts after
# x block 0 lands (~13us) instead of after the whole qk phase (~26us).
FLAG_WEAVE = os.environ.get("K_WEAVE", "1") == "1"
FLAG_QKCOPY_ACT = os.environ.get("K_QKCOPY_ACT", "0") == "1"
FLAG_MEMSET_ONES = os.environ.get("K_MEMSET_ONES", "1") == "1"
FLAG_OU_SPLIT = os.environ.get("K_OU_SPLIT", "0") == "1"
FLAG_SKEW2 = os.environ.get("K_SKEW2", "0") == "1"
# proj row-blocks >= this use split ACT/DVE copies + sync-only DMA triggers
PP_SPLIT_MT = int(os.environ.get("K_PP_SPLIT_MT", "8"))

_CACHED_NC = None


def _build_nc():
    from contextlib import ExitStack

    import concourse.bass as bass
    import concourse.mybir as mybir
    import concourse.tile as tile
    from concourse import bacc

    f32 = mybir.dt.float32
    bf16 = mybir.dt.bfloat16
    i16 = mybir.dt.int16
    AF = mybir.ActivationFunctionType
    ALU = mybir.AluOpType

    nc = bacc.Bacc("TRN2", target_bir_lowering=False, debug=False,
                   num_devices=NCORES)

    # per-core inputs (host pre-sharded / pre-transposed)
    xT = nc.dram_tensor("xT", [C, N], bf16, kind="ExternalInput")
    wqkT = nc.dram_tensor("wqkT", [C, 2 * CG], bf16, kind="ExternalInput")
    wvT = nc.dram_tensor("wvT", [C, CG], bf16, kind="ExternalInput")
    wpT = nc.dram_tensor("wpT", [CG, C], bf16, kind="ExternalInput")
    f16 = mybir.dt.float16
    yp = nc.dram_tensor("yp", [N, C], f16, kind="ExternalOutput")

    with tile.TileContext(nc) as tc:
        with ExitStack() as ctx:
            singles = ctx.enter_context(tc.tile_pool(name="singles", bufs=1))
            tmp = ctx.enter_context(tc.tile_pool(name="tmp", bufs=3))
            ps_big = ctx.enter_context(
                tc.tile_pool(name="ps_big", bufs=3, space="PSUM"))
            ps1 = ctx.enter_context(
                tc.tile_pool(name="ps1", bufs=2, space="PSUM"))
            dscratch = ctx.enter_context(
                tc.tile_pool(name="dscratch", bufs=2, space="DRAM"))

            # ---- persistent SBUF tensors -------------------------------
            xT_sb = singles.tile([P, KT, N], bf16)         # x^T (c on part)
            wqk_sb = singles.tile([P, KT, 2 * CG], bf16)   # q|k weight cols
            wv_sb = singles.tile([P, KT, CG], bf16)
            wp_sb = singles.tile([P, CG // P, C], bf16)
            qT_sb = singles.tile([P, HG // 2, N], bf16)    # q^T (d on part)
            kT_sb = singles.tile([P, HG // 2, N], bf16)    # k^T (d on part)
            v_sb = singles.tile([P, KV_CHUNKS, HG * VB], bf16)
            oT_sb = singles.tile([P, CG // P, N], bf16)    # normalized o^T

            # ---- load inputs ------------------------------------------
            xT_ap = xT.ap().rearrange("(g p) r -> p g r", p=P)
            wqk_ap = wqkT.ap().rearrange("(g p) o -> p g o", p=P)
            if FLAG_DMA_V3:
                # priority order: x block 0 + the k0/q0 weight columns +
                # wv first (everything unit 0 needs), then the rest.  x
                # goes out as 8 big [128,4,512] descriptors over three
                # queues (sync/gpsimd/vector) so the aggregate feed runs
                # at HBM rate instead of two queues' worth.
                xq = [nc.sync, nc.gpsimd]
                def x_block(nb, qoff):
                    for h in range(2):
                        xq[(qoff + h) % 2].dma_start(
                            xT_sb[:, 4 * h:4 * h + 4, nb * 512:(nb + 1) * 512],
                            xT_ap[:, 4 * h:4 * h + 4, nb * 512:(nb + 1) * 512])
                # block 0 at per-j granularity so the first k0 matmul only
                # waits on one 128KB transfer; later blocks as 4-chunk
                # descriptors (fewer triggers).
                for j in range(KT):
                    xq[j % 2].dma_start(
                        xT_sb[:, j, 0:512], xT_ap[:, j, 0:512])
                nc.scalar.dma_start(wqk_sb[:, 0:2, 0:256],
                                    wqk_ap[:, 0:2, 0:256])
                nc.scalar.dma_start(wqk_sb[:, 2:8, 0:256],
                                    wqk_ap[:, 2:8, 0:256])
                nc.scalar.dma_start(
                    wv_sb[:], wvT.ap().rearrange("(g p) o -> p g o", p=P))
                for nb in range(1, QT):
                    x_block(nb, 2 * nb)
                nc.scalar.dma_start(wqk_sb[:, :, 256:512],
                                    wqk_ap[:, :, 256:512])
                nc.scalar.dma_start(
                    wp_sb[:], wpT.ap().rearrange("(g p) o -> p g o", p=P))
            elif FLAG_WEAVE:
                # unit-0-first priority: the k0/q0 weight columns land
                # before the k1/q1 ones, wv before wp, and x streams
                # per-j nb-outer on the other two queues.
                nc.scalar.dma_start(wqk_sb[:, 0:2, 0:256],
                                    wqk_ap[:, 0:2, 0:256])
                for nb in range(QT):
                    for j in range(KT):
                        eng = nc.sync if j % 2 == 0 else nc.gpsimd
                        eng.dma_start(
                            xT_sb[:, j, nb * 512:(nb + 1) * 512],
                            xT_ap[:, j, nb * 512:(nb + 1) * 512])
                nc.scalar.dma_start(wqk_sb[:, 2:8, 0:256],
                                    wqk_ap[:, 2:8, 0:256])
                nc.scalar.dma_start(
                    wv_sb[:], wvT.ap().rearrange("(g p) o -> p g o", p=P))
                nc.scalar.dma_start(wqk_sb[:, :, 256:512],
                                    wqk_ap[:, :, 256:512])
                nc.scalar.dma_start(
                    wp_sb[:], wpT.ap().rearrange("(g p) o -> p g o", p=P))
            else:
                for j in range(KT):
                    nc.scalar.dma_start(wqk_sb[:, j, :], wqk_ap[:, j, :])
                for nb in range(QT):
                    for j in range(KT):
                        eng = nc.sync if j % 2 == 0 else nc.gpsimd
                        eng.dma_start(
                            xT_sb[:, j, nb * 512:(nb + 1) * 512],
                            xT_ap[:, j, nb * 512:(nb + 1) * 512])
                nc.scalar.dma_start(
                    wv_sb[:], wvT.ap().rearrange("(g p) o -> p g o", p=P))
                nc.scalar.dma_start(
                    wp_sb[:], wpT.ap().rearrange("(g p) o -> p g o", p=P))
            v_view = v_sb[:].rearrange("p c (h e) -> p c h e", e=VB)
            if FLAG_MEMSET_ONES:
                # only the per-head ones columns; data columns are fully
                # overwritten by the v copies.
                nc.vector.memset(v_view[:, :, :, D:D + 1], 1.0)
            else:
                nc.vector.memset(v_sb[:], 1.0)

            # ---- q^T / k^T / v projections -----------------------------
            # wqk columns: 0..CG-1 = q channels, CG..2CG-1 = k channels
            # nchunk outer so the first 512-token DMA batch feeds the whole
            # first j-loop; one pts tile per nchunk, rotating.
            # wqk column blocks (host order): m = 0:k-pair0, 1:q-pair0,
            # 2:k-pair1, 3:q-pair1
            def qk_mtile(m, nchunks=range(QT)):
                dst = kT_sb if m % 2 == 0 else qT_sb
                dm = m // 2
                for nchunk in nchunks:
                    pt = ps_big.tile([P, 1024], f32, tag="sc",
                                     name=f"pts{m}_{nchunk}")
                    for j in range(KT):
                        nc.tensor.matmul(
                            pt[:, 0:512],
                            wqk_sb[:, j, m * P:(m + 1) * P],
                            xT_sb[:, j, nchunk * 512:(nchunk + 1) * 512],
                            start=(j == 0), stop=(j == KT - 1))
                    if FLAG_QKCOPY_ACT:
                        # ACT has ~2us/g slack during unit 0 (no ou/pp
                        # copies yet) while the DVE is running Schraudolph
                        # exps + v copies there.
                        nc.scalar.copy(
                            out=dst[:, dm, nchunk * 512:(nchunk + 1) * 512],
                            in_=pt[:, 0:512])
                    else:
                        nc.vector.tensor_copy(
                            out=dst[:, dm, nchunk * 512:(nchunk + 1) * 512],
                            in_=pt[:, 0:512])

            def v_rtile(rt):
                pt = ps_big.tile([P, 1024], f32, tag="sc")
                for j in range(KT):
                    nc.tensor.matmul(
                        pt[:, :CG], xT_sb[:, j, rt * P:(rt + 1) * P],
                        wv_sb[:, j, :], start=(j == 0), stop=(j == KT - 1))
                nc.vector.tensor_copy(
                    out=v_view[:, rt, :, :D],
                    in_=pt[:, :CG].rearrange("p (h d) -> p h d", d=D))

            # emission order minimizes the PE lead-in before the first
            # score matmuls: k/q of pair 0 first (q only needs its first
            # 512-token block), then the rest woven before pair 1's units.
            if FLAG_WEAVE:
                # only what unit 0 group 0 strictly needs is emitted ahead
                # of the unit loop; the rest of the qk projections become
                # fillers popped during units 0-1, overlapping the x DMA.
                qk_mtile(0, [0])   # k pair 0, kv 0:512
                qk_mtile(1, [0])   # q pair 0, tokens 0:512
                weave0 = [((0, 1), 2), ((0, 2), 3), ((0, 3), 4),
                          ((2, 0), 4), ((2, 1), 5), ((2, 2), 5),
                          ((2, 3), 6), ((3, 0), 7)]
                pair0_rest = [(1, 1), (1, 2), (1, 3),
                              (3, 1), (3, 2), (3, 3)]
            else:
                qk_mtile(0)            # k pair 0 (all 2048 kv)
                qk_mtile(1, [0])       # q pair 0, tokens 0:512 only
                weave0 = []
                pair0_rest = [(1, 1), (1, 2), (1, 3)]
                qk_mtile(2)            # k pair 1
                qk_mtile(3)            # q pair 1

            # PE filler queue: closures emitted one per attention group
            # iteration, each no earlier than `delay` iterations after
            # being enqueued (lets upstream DMA/engine chains complete
            # before the PE hits the dependent matmuls).
            fillers = []           # list of (ready_iteration, closure)
            it_counter = [0]

            def push_filler(fn, delay=0):
                fillers.append((it_counter[0] + delay, fn))

            def pop_filler():
                it_counter[0] += 1
                if fillers and fillers[0][0] <= it_counter[0]:
                    fillers.pop(0)[1]()

            # ---- attention: software-pipelined emission ----------------
            # Units are (pair, qt), qt-major so each 512-row block of the
            # output projection can be emitted as PE filler right after its
            # two units finish.  Within the global stream, the A@V matmuls
            # for group t are emitted AFTER the score matmuls of group t+1:
            # the PE is in-order, so this one-group skew keeps it from
            # stalling on the exp (ACT/DVE) results.
            GROUP = 2  # kv chunks per exp batch (PSUM tile = 2 banks)
            NGRP = KV_CHUNKS // GROUP

            # deferred normalize multiplies: appended at a unit's end,
            # emitted gradually (one readiness check per g iteration) so
            # (a) the gpsimd broadcast has ~2 iterations to complete before
            # the DVE hits the mul, and (b) with FLAG_MUL_PSUM the o_acc
            # PSUM banks are released early in the next unit, before the
            # unit after that re-allocates them.
            git = [0]              # global (u,g) iteration counter
            pending_muls = []      # (ready_iteration, closure)

            def push_mul(fn, delay=2):
                pending_muls.append((git[0] + delay, fn))

            def step_muls():
                while pending_muls and pending_muls[0][0] <= git[0]:
                    pending_muls.pop(0)[1]()

            def flush_muls(keep=0):
                while len(pending_muls) > keep:
                    pending_muls.pop(0)[1]()

            def normalize_pair(o_acc_pair, pair, qt):
                # Stage both unnormalized accumulators to SBUF immediately so
                # the PSUM banks free for the next unit's A@V.  v3 chain:
                # one ACT copy evacuates data+denominator together ([65,512],
                # the den rides along as row 64), DVE computes 1/den on the
                # single row ([1,512], cheap), GPSIMD broadcasts the
                # reciprocal, DVE multiplies.  vs v2 this drops the separate
                # [1,512] DVE den copy and shrinks the reciprocal from
                # [64,512] to [1,512].
                for hx, po in ((0, 0), (1, D)):
                    # o_acc rows: 0..63 = data, 64 = denominator (ones col)
                    den_t = tmp.tile([1, 512], f32, tag="dent", bufs=4,
                                     name=f"dent{pair}_{qt}_{hx}")
                    if FLAG_DEN_ACT:
                        nc.scalar.copy(out=den_t[:],
                                       in_=o_acc_pair[hx][D:D + 1])
                    else:
                        nc.vector.tensor_copy(out=den_t[:],
                                              in_=o_acc_pair[hx][D:D + 1])
                    if FLAG_MUL_PSUM:
                        # the deferred mul reads the accumulator straight
                        # from PSUM; no ACT staging copy.  The PSUM banks
                        # are held until the mul runs (one unit later),
                        # which the 2-deep ps1 rotation absorbs.
                        ou = o_acc_pair[hx]
                    else:
                        ou = tmp.tile([D, 512], f32, tag="ou", bufs=4,
                                      name=f"ou{pair}_{qt}_{hx}")
                        if FLAG_OU_SPLIT and hx == 1:
                            # stage the two heads on different engines so
                            # both PSUM accumulators free concurrently: the
                            # next unit's first A@V has a WAR on these.
                            nc.vector.tensor_copy(out=ou[:],
                                                  in_=o_acc_pair[hx][:D])
                        else:
                            nc.scalar.copy(out=ou[:], in_=o_acc_pair[hx][:D])
                    if FLAG_DIV:
                        # broadcast the raw denominator; the deferred DVE op
                        # is a single tensor_tensor divide — no reciprocal.
                        bc_sb = tmp.tile([D, 512], f32, tag="bcsb", bufs=4,
                                         name=f"bcsb{pair}_{qt}_{hx}")
                        nc.gpsimd.partition_broadcast(bc_sb[:], den_t[:],
                                                      channels=D)

                        def mul(ou=ou, bc_sb=bc_sb, po=po, pair=pair, qt=qt):
                            nc.vector.tensor_tensor(
                                out=oT_sb[po:po + D, pair,
                                          qt * 512:(qt + 1) * 512],
                                in0=ou[:D], in1=bc_sb[:],
                                op=ALU.divide)
                    elif FLAG_NORM_V3:
                        # reciprocal on the single den row BEFORE the
                        # broadcast: [1,512] instead of [64,512] on DVE.
                        # custom DVE/GPSIMD ops only read partition-0 tiles.
                        rec1 = tmp.tile([1, 512], f32, tag="rec1", bufs=4,
                                        name=f"rec1{pair}_{qt}_{hx}")
                        nc.vector.reciprocal_approx_fast(out=rec1[:],
                                                         in_=den_t[:])
                        bc_sb = tmp.tile([D, 512], f32, tag="bcsb", bufs=4,
                                         name=f"bcsb{pair}_{qt}_{hx}")
                        nc.gpsimd.partition_broadcast(bc_sb[:], rec1[:],
                                                      channels=D)

                        def mul(ou=ou, bc_sb=bc_sb, po=po, pair=pair, qt=qt):
                            nc.vector.tensor_mul(
                                out=oT_sb[po:po + D, pair,
                                          qt * 512:(qt + 1) * 512],
                                in0=ou[:D], in1=bc_sb[:])
                    else:
                        bc_sb = tmp.tile([D, 512], f32, tag="bcsb", bufs=4,
                                         name=f"bcsb{pair}_{qt}_{hx}")
                        nc.gpsimd.partition_broadcast(bc_sb[:], den_t[:],
                                                      channels=D)

                        def mul(ou=ou, bc_sb=bc_sb, po=po, pair=pair, qt=qt,
                                hx=hx):
                            rec_bc = tmp.tile([D, 512], f32, tag="recbc",
                                              bufs=4,
                                              name=f"recbc{pair}_{qt}_{hx}")
                            nc.vector.reciprocal_approx_fast(out=rec_bc[:],
                                                             in_=bc_sb[:])
                            nc.vector.tensor_mul(
                                out=oT_sb[po:po + D, pair,
                                          qt * 512:(qt + 1) * 512],
                                in0=ou[:D], in1=rec_bc[:])
                    push_mul(mul)

            def proj_mt(mt):
                # partial output projection for rows [mt*128, mt*128+128)
                pp = ps_big.tile([P, 1024], f32, tag="sc", name=f"pp{mt}")
                for nh in range(2):
                    for j in range(CG // P):
                        nc.tensor.matmul(
                            pp[:, nh * 512:nh * 512 + 512],
                            oT_sb[:, j, mt * P:(mt + 1) * P],
                            wp_sb[:, j, nh * 512:(nh + 1) * 512],
                            start=(j == 0), stop=(j == CG // P - 1))
                ysb = tmp.tile([P, 1024], f16, tag="ysb", name=f"ysb{mt}")
                # NEVER the gpsimd queue: a y-DMA trigger there waits on the
                # proj chain and would wedge partition_broadcast behind it.
                if FLAG_PP_MERGE and mt < PP_SPLIT_MT:
                    # one [128,1024] ACT op (spans both PSUM banks): the
                    # ~215ns per-op ACT overhead is paid once instead of
                    # twice.  qt2/qt3 keep split halves for tail overlap.
                    nc.scalar.copy(out=ysb[:], in_=pp[:])
                    for h2 in range(2):
                        cs = slice(h2 * 512, h2 * 512 + 512)
                        eng = (nc.sync, nc.scalar)[(2 * mt + h2) % 2]
                        eng.dma_start(yp.ap()[mt * P:(mt + 1) * P, cs],
                                      ysb[:, cs])
                else:
                    # tail projections (qt2+qt3 run in unit 7 / the drain):
                    # the ACT queue is the end-of-kernel serializer (exp g7
                    # + ou copies + pp copies back-to-back), so split the
                    # PSUM evacuation halves across ACT and the by-then
                    # idle DVE, and put BOTH yp DMA triggers on the idle
                    # sync queue instead of costing ~0.6us each on ACT.
                    for h2 in range(2):
                        cs = slice(h2 * 512, h2 * 512 + 512)
                        if h2 == 1:
                            nc.vector.tensor_copy(out=ysb[:, cs],
                                                  in_=pp[:, cs])
                        else:
                            nc.scalar.copy(out=ysb[:, cs], in_=pp[:, cs])
                        nc.sync.dma_start(yp.ap()[mt * P:(mt + 1) * P, cs],
                                          ysb[:, cs])

            units = [(pair, qt) for qt in range(QT) for pair in range(HG // 2)]
            o_accs_u = {}
            pending = []        # (u, g, exs) whose A@V is not yet emitted
            # SKEW2: A@V for group g is emitted after the scores of g+2
            # (not g+1): the ex pool (6 bufs = 3 groups) exactly covers it,
            # every A@V gets a full extra group of exp headroom, and the
            # next unit's first A@V no longer races the previous unit's
            # ou staging copies on the reused ps1 banks.
            SKEW = 2 if FLAG_SKEW2 else 1

            def emit_av(u, g, exs):
                pair, qt = units[u]
                for i in range(GROUP):
                    r = g * GROUP + i
                    for hx, h in ((0, 2 * pair), (1, 2 * pair + 1)):
                        nc.tensor.matmul(
                            o_accs_u[u][hx][:VB, :],
                            v_sb[:, r, h * VB:(h + 1) * VB],
                            exs[hx][:, i * 512:i * 512 + 512],
                            start=(r == 0), stop=(r == KV_CHUNKS - 1))
                if g == NGRP - 1:
                    # muls land via push_mul/step_muls: two iterations of
                    # deferral hide the gpsimd broadcast round-trip, and
                    # the gradual flush keeps the emission ahead of the
                    # ps1-pool reuse two units later.
                    normalize_pair(o_accs_u[u], pair, qt)
                    del o_accs_u[u]
                    if pair == 1:
                        # qt2's projection is deliberately delayed to land
                        # in unit 7 / the final drain: it keeps the PE busy
                        # (and out of the low-clock p-state) while the last
                        # unit's normalize chain runs, instead of idling
                        # 4us and then running the tail proj at 1.2GHz.
                        base = 14 if qt == 2 else 6
                        for mt4 in range(4):
                            push_filler(
                                (lambda mt: lambda: proj_mt(mt))(qt * 4 + mt4),
                                delay=base + 2 * mt4)

            for u, (pair, qt) in enumerate(units):
                qs = slice(qt * 512, (qt + 1) * 512)
                o_accs_u[u] = [ps1.tile([P, 512], f32, tag="ps1",
                                        name=f"oacc{pair}_{qt}_{i}")
                               for i in range(2)]
                if u == 0:
                    for (m, nchunk), dly in weave0:
                        push_filler(
                            (lambda a, b: lambda: qk_mtile(a, [b]))(m, nchunk),
                            delay=dly)
                if u == 1:
                    for m, nchunk in pair0_rest:
                        push_filler(
                            (lambda a, b: lambda: qk_mtile(a, [b]))(m, nchunk))
                for g in range(NGRP):
                    git[0] += 1
                    scs = [ps_big.tile([P, 1024], f32, tag="sc",
                                       name=f"sc{pair}_{qt}_{g}_{i}")
                           for i in range(2)]
                    # 64x64-mode score matmuls: per 512-wide round, 4
                    # concurrent tiles = (head pair) x (kv half of chunk).
                    for i in range(GROUP):
                        r = g * GROUP + i
                        if FLAG_SCORES_TILED:
                            for hx, po in ((0, 0), (1, D)):
                                for kvh in (0, 1):
                                    nc.tensor.matmul(
                                        scs[hx][kvh * 64:kvh * 64 + 64,
                                                i * 512:i * 512 + 512],
                                        kT_sb[po:po + D, pair,
                                              r * P + kvh * 64:
                                              r * P + kvh * 64 + 64],
                                        qT_sb[po:po + D, pair, qs],
                                        start=True, stop=True,
                                        tile_position=(po, kvh * 64))
                        else:
                            for hx, po in ((0, 0), (1, D)):
                                nc.tensor.matmul(
                                    scs[hx][:, i * 512:i * 512 + 512],
                                    kT_sb[po:po + D, pair,
                                          r * P:(r + 1) * P],
                                    qT_sb[po:po + D, pair, qs],
                                    start=True, stop=True)
                    # exp: head A on ACT (table exp); head B split between
                    # DVE (Schraudolph bit-trick, cols 0:EXP_SPLIT) and ACT
                    # (exact exp on the tail cols).  The split balances the
                    # two engines' per-group load so the A@V matmuls never
                    # wait on the slower one; per-head softmax cancels the
                    # Schraudolph systematic scale error.
                    ex0 = tmp.tile([P, 1024], bf16, tag="ex", bufs=6,
                                   name=f"ex{pair}_{qt}_{g}_0")
                    nc.scalar.activation(ex0[:], scs[0][:], AF.Exp,
                                         scale=SCALE)
                    if FLAG_EXP_DVE:
                        ex1_i = tmp.tile([P, 1024], i16, tag="exb", bufs=6,
                                         name=f"ex{pair}_{qt}_{g}_1")
                        if FLAG_EXP_CHUNK:
                            # one DVE op per kv chunk: the A@V matmuls for
                            # chunk i=0 only wait on the first half, not
                            # the whole [128,1024] op.
                            for ic in range(GROUP):
                                cs = slice(ic * 512, ic * 512 + 512)
                                nc.vector.tensor_scalar(
                                    out=ex1_i[:, cs], in0=scs[1][:, cs],
                                    scalar1=EXP_A, scalar2=EXP_B,
                                    op0=ALU.mult, op1=ALU.add)
                        else:
                            nc.vector.tensor_scalar(
                                out=ex1_i[:, 0:EXP_SPLIT],
                                in0=scs[1][:, 0:EXP_SPLIT],
                                scalar1=EXP_A, scalar2=EXP_B,
                                op0=ALU.mult, op1=ALU.add)
                            if EXP_SPLIT < 1024:
                                nc.scalar.activation(
                                    ex1_i[:, EXP_SPLIT:1024].bitcast(bf16),
                                    scs[1][:, EXP_SPLIT:1024], AF.Exp,
                                    scale=SCALE)
                        ex1 = ex1_i[:].bitcast(bf16)
                    else:
                        ex1_t = tmp.tile([P, 1024], bf16, tag="exb2", bufs=6,
                                         name=f"ex{pair}_{qt}_{g}_1")
                        nc.scalar.activation(ex1_t[:], scs[1][:], AF.Exp,
                                             scale=SCALE)
                        ex1 = ex1_t[:]
                    exs = [ex0, ex1]
                    # fillers/muls AFTER the exps: their PE matmuls deepen
                    # the score->A@V skew, and their DVE/ACT side ops queue
                    # behind this group's exps instead of delaying them.
                    if u == 0:
                        v_rtile(2 * g)
                        v_rtile(2 * g + 1)
                        if FLAG_WEAVE:
                            # two pop slots per g: the x-arrival-gated qk
                            # fillers drain without head-of-line stalls
                            pop_filler()
                            pop_filler()
                    else:
                        pop_filler()
                    step_muls()
                    pending.append((u, g, exs))
                    while len(pending) > SKEW:
                        emit_av(*pending.pop(0))
            while pending:
                emit_av(*pending.pop(0))
            flush_muls(keep=0)
            while fillers:
                fillers.pop(0)[1]()

    nc.compile()
    return nc


def _host_prep(x, w_qkv, w_proj, b_proj):
    import ml_dtypes
    bf16 = ml_dtypes.bfloat16
    wqkvT = np.ascontiguousarray(w_qkv.T).astype(bf16)   # [C, 3C]
    wpT_full = np.ascontiguousarray(w_proj.T).astype(bf16)  # [C(in), C(out)]
    in_maps = []
    for c in range(NCORES):
        b, g = divmod(c, GROUPS)
        qcols = wqkvT[:, CG * g:CG * (g + 1)]
        kcols = wqkvT[:, C + CG * g:C + CG * (g + 1)]
        vcols = wqkvT[:, 2 * C + CG * g:2 * C + CG * (g + 1)]
        # column order [k_pair0 | q_pair0 | k_pair1 | q_pair1] matches the
        # kernel's matmul emission order (see qk_mtile)
        wqk = np.ascontiguousarray(np.concatenate(
            [kcols[:, :128], qcols[:, :128],
             kcols[:, 128:], qcols[:, 128:]], axis=1))
        wv = np.ascontiguousarray(vcols)
        wp = np.ascontiguousarray(wpT_full[CG * g:CG * (g + 1), :])
        xTv = np.ascontiguousarray(x[b].T).astype(bf16)
        in_maps.append({"xT": xTv, "wqkT": wqk, "wvT": wv, "wpT": wp})
    return in_maps


def run(inputs, trace=False, nc=None):
    """Build (or reuse) the program, run on 8 cores, return (y, results)."""
    global _CACHED_NC
    from concourse.bass_utils import run_bass_kernel_spmd
    if nc is None:
        if _CACHED_NC is None:
            _CACHED_NC = _build_nc()
        nc = _CACHED_NC
    in_maps = _host_prep(**inputs)
    res = run_bass_kernel_spmd(nc, in_maps, core_ids=list(range(NCORES)),
                               trace=trace)
    bias = np.asarray(inputs["b_proj"], np.float32)
    out = np.empty((B, N, C), np.float32)
    for b in range(B):
        acc = res.results[b * GROUPS]["yp"].astype(np.float32)
        for g in range(1, GROUPS):
            acc = acc + res.results[b * GROUPS + g]["yp"]
        out[b] = acc + bias
    return out, res


def kernel(x, w_qkv, w_proj, b_proj):
    out, _ = run({"x": np.asarray(x), "w_qkv": np.asarray(w_qkv),
                  "w_proj": np.asarray(w_proj), "b_proj": np.asarray(b_proj)})
    return out



# revision 58
# speedup vs baseline: 1.0064x; 1.0064x over previous
"""Trainium2 Bass kernel for nn_Attention (dense transformer MHA block).

Reference computation (fp32):
    qkv = x @ w_qkv.T            # [B,N,3C]
    q,k,v per head; scores = q k^T / sqrt(D); attn = softmax(scores)
    o = attn @ v;  y = o @ w_proj.T + b_proj

Sharding over 8 NeuronCores (data-parallel over batch x tensor-parallel over
heads): core c -> (batch b = c//4, head group g = c%4, heads 4g..4g+3).
Each core computes q/k/v for its 4 heads over the full 2048-token sequence,
runs attention locally, and multiplies by its row-slice of w_proj, producing
a PARTIAL output [2048, 1024].  The 4 partials per batch are summed on the
host (numpy) together with the bias — no device collectives.

Perf structure (v7, ~209us vs 223us for v2):
  - score matmuls run in 64x64 array-tiling mode: 4 concurrent tiles
    (2 heads x 2 kv-halves) per 512-wide round, ~2x faster than the
    untiled 64-contraction matmuls.
  - exp is split across engines: head A of each pair uses the ACT table
    exp; head B uses a Schraudolph bit-trick exp on the DVE
    (y_i16 = round(s*A + B) bitcast as bf16 == exp(s*scale) within ~4%).
    The per-head softmax normalization cancels each head's systematic
    scale error.
  - A@V keeps the ones-column trick (V gets a 65th column of ones so the
    same matmul accumulates the softmax denominator) - provably optimal:
    A@V is rhs-stream-bound (E enters the PE at 128 lanes/cycle), so any
    separate denominator matmul would re-stream all exp columns.
  - WEAVE startup: only k0/q0 of the first 512-token block precede the
    unit loop; the other 8 qk projection tiles become PE fillers popped
    through unit 0 (2 slots per group), so attention starts as soon as
    x block 0 + the k0/q0 weight columns land (~13us) instead of after
    the full qk phase (~26us), overlapping the remaining x DMA.
  - per-iteration emission order [scores -> exps -> fillers -> A@V]:
    filler matmuls deepen the score->A@V skew on the in-order PE, and
    filler DVE copies queue behind the exps instead of delaying them.
  - normalize: den row staged [1,512] (DVE), 1/den before broadcast
    ([1,512] DVE reciprocal, GPSIMD broadcast of the reciprocal), muls
    deferred ~2 iterations via a readiness queue.  Custom DVE/GPSIMD ops
    (reciprocal_approx_fast, partition_broadcast) MUST read partition-0
    tiles - feeding them partition-offset-64 APs returns garbage (NaN).
  - proj evacuation: one [128,1024] ACT copy per row-block (spans both
    PSUM banks; ACT per-op overhead ~215ns makes two half copies
    slower); final query tile keeps split ACT/DVE halves for tail
    overlap.  qt2's projection is delay-pushed so ~half of it drains
    after the last A@V, keeping the PE busy (and out of the low-clock
    p-state) while the final normalize chain runs.
  - engine-queue discipline (measured, do not "rebalance" casually):
    moving den copies or qk-tile copies to ACT puts DMA/matmul-gated
    ops in front of the exps on an in-order queue and loses 2-37us.
    tensor_tensor(divide) is rejected by codegen (s3s3d3_tt_valid_op);
    fp8 anywhere fails the 2e-2 gate (scores err ~0.12 absolute).
"""

import numpy as np

B, N, C = 2, 2048, 1024
H, D = 16, 64
NCORES = 8
GROUPS = 4              # head groups (tensor-parallel)
HG = H // GROUPS        # 4 heads per core
CG = HG * D             # 256 channels per core
P = 128
KT = C // P             # 8 contraction subtiles for C=1024
KV_CHUNKS = N // P      # 16 key/value chunks of 128 rows
QT = N // 512           # 4 query tiles of 512
VB = D + 1              # v block width incl. ones column (65)
SCALE = 1.0 / float(np.sqrt(D))
# Schraudolph exp constants (bf16 bit-trick on DVE): for scores s (pre-scale),
# exp(s*SCALE) ~= bitcast_bf16(int16(s*EXP_A + EXP_B)).  The -7.63 centers the
# sawtooth approximation error; the per-head softmax cancels the global scale.
EXP_A = 128.0 * 1.4426950408889634 * SCALE
EXP_B = 127.0 * 128.0 - 7.63

import os
FLAG_SCORES_TILED = os.environ.get("K_SCORES_TILED", "1") == "1"
FLAG_EXP_DVE = os.environ.get("K_EXP_DVE", "1") == "1"
FLAG_MUL_GPSIMD = os.environ.get("K_MUL_GPSIMD", "1") == "1"
# v3 knobs
EXP_SPLIT = int(os.environ.get("K_EXP_SPLIT", "1024"))  # DVE cols of scs[1]
FLAG_EXP_CHUNK = os.environ.get("K_EXP_CHUNK", "0") == "1"  # DVE exp per-chunk
# muls read o_acc straight from PSUM: saves the ACT ou copies but the
# deferred-mul WAR on the 2-deep ps1 pool serializes unit boundaries.
FLAG_MUL_PSUM = os.environ.get("K_MUL_PSUM", "0") == "1"
# tensor_tensor divide: REJECTED by codegen (s3s3d3_tt_valid_op) — fp32
# divide is not a valid DVE tensor_tensor op.  Keep recip+mul.
FLAG_DIV = os.environ.get("K_DIV", "0") == "1"
FLAG_DEN_ACT = os.environ.get("K_DEN_ACT", "0") == "1"  # den copy on ACT
FLAG_PP_MERGE = os.environ.get("K_PP_MERGE", "1") == "1"    # 1-op proj evac
FLAG_NORM_V3 = os.environ.get("K_NORM_V3", "1") == "1"
FLAG_DMA_V3 = os.environ.get("K_DMA_V3", "0") == "1"
# v7: weave k0b1..3/k1/q1 into unit 0 as fillers so attention starts after
# x block 0 lands (~13us) instead of after the whole qk phase (~26us).
FLAG_WEAVE = os.environ.get("K_WEAVE", "1") == "1"
FLAG_QKCOPY_ACT = os.environ.get("K_QKCOPY_ACT", "0") == "1"
FLAG_MEMSET_ONES = os.environ.get("K_MEMSET_ONES", "1") == "1"
FLAG_OU_SPLIT = os.environ.get("K_OU_SPLIT", "0") == "1"
FLAG_SKEW2 = os.environ.get("K_SKEW2", "0") == "1"
# proj row-blocks >= this use split ACT/DVE copies + sync-only DMA triggers
PP_SPLIT_MT = int(os.environ.get("K_PP_SPLIT_MT", "12"))

_CACHED_NC = None


def _build_nc():
    from contextlib import ExitStack

    import concourse.bass as bass
    import concourse.mybir as mybir
    import concourse.tile as tile
    from concourse import bacc

    f32 = mybir.dt.float32
    bf16 = mybir.dt.bfloat16
    i16 = mybir.dt.int16
    AF = mybir.ActivationFunctionType
    ALU = mybir.AluOpType

    nc = bacc.Bacc("TRN2", target_bir_lowering=False, debug=False,
                   num_devices=NCORES)

    # per-core inputs (host pre-sharded / pre-transposed)
    xT = nc.dram_tensor("xT", [C, N], bf16, kind="ExternalInput")
    wqkT = nc.dram_tensor("wqkT", [C, 2 * CG], bf16, kind="ExternalInput")
    wvT = nc.dram_tensor("wvT", [C, CG], bf16, kind="ExternalInput")
    wpT = nc.dram_tensor("wpT", [CG, C], bf16, kind="ExternalInput")
    f16 = mybir.dt.float16
    yp = nc.dram_tensor("yp", [N, C], f16, kind="ExternalOutput")

    with tile.TileContext(nc) as tc:
        with ExitStack() as ctx:
            singles = ctx.enter_context(tc.tile_pool(name="singles", bufs=1))
            tmp = ctx.enter_context(tc.tile_pool(name="tmp", bufs=3))
            ps_big = ctx.enter_context(
                tc.tile_pool(name="ps_big", bufs=3, space="PSUM"))
            ps1 = ctx.enter_context(
                tc.tile_pool(name="ps1", bufs=2, space="PSUM"))
            dscratch = ctx.enter_context(
                tc.tile_pool(name="dscratch", bufs=2, space="DRAM"))

            # ---- persistent SBUF tensors -------------------------------
            xT_sb = singles.tile([P, KT, N], bf16)         # x^T (c on part)
            wqk_sb = singles.tile([P, KT, 2 * CG], bf16)   # q|k weight cols
            wv_sb = singles.tile([P, KT, CG], bf16)
            wp_sb = singles.tile([P, CG // P, C], bf16)
            qT_sb = singles.tile([P, HG // 2, N], bf16)    # q^T (d on part)
            kT_sb = singles.tile([P, HG // 2, N], bf16)    # k^T (d on part)
            v_sb = singles.tile([P, KV_CHUNKS, HG * VB], bf16)
            oT_sb = singles.tile([P, CG // P, N], bf16)    # normalized o^T

            # ---- load inputs ------------------------------------------
            xT_ap = xT.ap().rearrange("(g p) r -> p g r", p=P)
            wqk_ap = wqkT.ap().rearrange("(g p) o -> p g o", p=P)
            if FLAG_DMA_V3:
                # priority order: x block 0 + the k0/q0 weight columns +
                # wv first (everything unit 0 needs), then the rest.  x
                # goes out as 8 big [128,4,512] descriptors over three
                # queues (sync/gpsimd/vector) so the aggregate feed runs
                # at HBM rate instead of two queues' worth.
                xq = [nc.sync, nc.gpsimd]
                def x_block(nb, qoff):
                    for h in range(2):
                        xq[(qoff + h) % 2].dma_start(
                            xT_sb[:, 4 * h:4 * h + 4, nb * 512:(nb + 1) * 512],
                            xT_ap[:, 4 * h:4 * h + 4, nb * 512:(nb + 1) * 512])
                # block 0 at per-j granularity so the first k0 matmul only
                # waits on one 128KB transfer; later blocks as 4-chunk
                # descriptors (fewer triggers).
                for j in range(KT):
                    xq[j % 2].dma_start(
                        xT_sb[:, j, 0:512], xT_ap[:, j, 0:512])
                nc.scalar.dma_start(wqk_sb[:, 0:2, 0:256],
                                    wqk_ap[:, 0:2, 0:256])
                nc.scalar.dma_start(wqk_sb[:, 2:8, 0:256],
                                    wqk_ap[:, 2:8, 0:256])
                nc.scalar.dma_start(
                    wv_sb[:], wvT.ap().rearrange("(g p) o -> p g o", p=P))
                for nb in range(1, QT):
                    x_block(nb, 2 * nb)
                nc.scalar.dma_start(wqk_sb[:, :, 256:512],
                                    wqk_ap[:, :, 256:512])
                nc.scalar.dma_start(
                    wp_sb[:], wpT.ap().rearrange("(g p) o -> p g o", p=P))
            elif FLAG_WEAVE:
                # unit-0-first priority: the k0/q0 weight columns land
                # before the k1/q1 ones, wv before wp, and x streams
                # per-j nb-outer on the other two queues.
                nc.scalar.dma_start(wqk_sb[:, 0:2, 0:256],
                                    wqk_ap[:, 0:2, 0:256])
                for nb in range(QT):
                    for j in range(KT):
                        eng = nc.sync if j % 2 == 0 else nc.gpsimd
                        eng.dma_start(
                            xT_sb[:, j, nb * 512:(nb + 1) * 512],
                            xT_ap[:, j, nb * 512:(nb + 1) * 512])
                nc.scalar.dma_start(wqk_sb[:, 2:8, 0:256],
                                    wqk_ap[:, 2:8, 0:256])
                nc.scalar.dma_start(
                    wv_sb[:], wvT.ap().rearrange("(g p) o -> p g o", p=P))
                nc.scalar.dma_start(wqk_sb[:, :, 256:512],
                                    wqk_ap[:, :, 256:512])
                nc.scalar.dma_start(
                    wp_sb[:], wpT.ap().rearrange("(g p) o -> p g o", p=P))
            else:
                for j in range(KT):
                    nc.scalar.dma_start(wqk_sb[:, j, :], wqk_ap[:, j, :])
                for nb in range(QT):
                    for j in range(KT):
                        eng = nc.sync if j % 2 == 0 else nc.gpsimd
                        eng.dma_start(
                            xT_sb[:, j, nb * 512:(nb + 1) * 512],
                            xT_ap[:, j, nb * 512:(nb + 1) * 512])
                nc.scalar.dma_start(
                    wv_sb[:], wvT.ap().rearrange("(g p) o -> p g o", p=P))
                nc.scalar.dma_start(
                    wp_sb[:], wpT.ap().rearrange("(g p) o -> p g o", p=P))
            v_view = v_sb[:].rearrange("p c (h e) -> p c h e", e=VB)
            if FLAG_MEMSET_ONES:
                # only the per-head ones columns; data columns are fully
                # overwritten by the v copies.
                nc.vector.memset(v_view[:, :, :, D:D + 1], 1.0)
            else:
                nc.vector.memset(v_sb[:], 1.0)

            # ---- q^T / k^T / v projections -----------------------------
            # wqk columns: 0..CG-1 = q channels, CG..2CG-1 = k channels
            # nchunk outer so the first 512-token DMA batch feeds the whole
            # first j-loop; one pts tile per nchunk, rotating.
            # wqk column blocks (host order): m = 0:k-pair0, 1:q-pair0,
            # 2:k-pair1, 3:q-pair1
            def qk_mtile(m, nchunks=range(QT)):
                dst = kT_sb if m % 2 == 0 else qT_sb
                dm = m // 2
                for nchunk in nchunks:
                    pt = ps_big.tile([P, 1024], f32, tag="sc",
                                     name=f"pts{m}_{nchunk}")
                    for j in range(KT):
                        nc.tensor.matmul(
                            pt[:, 0:512],
                            wqk_sb[:, j, m * P:(m + 1) * P],
                            xT_sb[:, j, nchunk * 512:(nchunk + 1) * 512],
                            start=(j == 0), stop=(j == KT - 1))
                    if FLAG_QKCOPY_ACT:
                        # ACT has ~2us/g slack during unit 0 (no ou/pp
                        # copies yet) while the DVE is running Schraudolph
                        # exps + v copies there.
                        nc.scalar.copy(
                            out=dst[:, dm, nchunk * 512:(nchunk + 1) * 512],
                            in_=pt[:, 0:512])
                    else:
                        nc.vector.tensor_copy(
                            out=dst[:, dm, nchunk * 512:(nchunk + 1) * 512],
                            in_=pt[:, 0:512])

            def v_rtile(rt):
                pt = ps_big.tile([P, 1024], f32, tag="sc")
                for j in range(KT):
                    nc.tensor.matmul(
                        pt[:, :CG], xT_sb[:, j, rt * P:(rt + 1) * P],
                        wv_sb[:, j, :], start=(j == 0), stop=(j == KT - 1))
                nc.vector.tensor_copy(
                    out=v_view[:, rt, :, :D],
                    in_=pt[:, :CG].rearrange("p (h d) -> p h d", d=D))

            # emission order minimizes the PE lead-in before the first
            # score matmuls: k/q of pair 0 first (q only needs its first
            # 512-token block), then the rest woven before pair 1's units.
            if FLAG_WEAVE:
                # only what unit 0 group 0 strictly needs is emitted ahead
                # of the unit loop; the rest of the qk projections become
                # fillers popped during units 0-1, overlapping the x DMA.
                qk_mtile(0, [0])   # k pair 0, kv 0:512
                qk_mtile(1, [0])   # q pair 0, tokens 0:512
                weave0 = [((0, 1), 2), ((0, 2), 3), ((0, 3), 4),
                          ((2, 0), 4), ((2, 1), 5), ((2, 2), 5),
                          ((2, 3), 6), ((3, 0), 7)]
                pair0_rest = [(1, 1), (1, 2), (1, 3),
                              (3, 1), (3, 2), (3, 3)]
            else:
                qk_mtile(0)            # k pair 0 (all 2048 kv)
                qk_mtile(1, [0])       # q pair 0, tokens 0:512 only
                weave0 = []
                pair0_rest = [(1, 1), (1, 2), (1, 3)]
                qk_mtile(2)            # k pair 1
                qk_mtile(3)            # q pair 1

            # PE filler queue: closures emitted one per attention group
            # iteration, each no earlier than `delay` iterations after
            # being enqueued (lets upstream DMA/engine chains complete
            # before the PE hits the dependent matmuls).
            fillers = []           # list of (ready_iteration, closure)
            it_counter = [0]

            def push_filler(fn, delay=0):
                fillers.append((it_counter[0] + delay, fn))

            def pop_filler():
                it_counter[0] += 1
                if fillers and fillers[0][0] <= it_counter[0]:
                    fillers.pop(0)[1]()

            # ---- attention: software-pipelined emission ----------------
            # Units are (pair, qt), qt-major so each 512-row block of the
            # output projection can be emitted as PE filler right after its
            # two units finish.  Within the global stream, the A@V matmuls
            # for group t are emitted AFTER the score matmuls of group t+1:
            # the PE is in-order, so this one-group skew keeps it from
            # stalling on the exp (ACT/DVE) results.
            GROUP = 2  # kv chunks per exp batch (PSUM tile = 2 banks)
            NGRP = KV_CHUNKS // GROUP

            # deferred normalize multiplies: appended at a unit's end,
            # emitted gradually (one readiness check per g iteration) so
            # (a) the gpsimd broadcast has ~2 iterations to complete before
            # the DVE hits the mul, and (b) with FLAG_MUL_PSUM the o_acc
            # PSUM banks are released early in the next unit, before the
            # unit after that re-allocates them.
            git = [0]              # global (u,g) iteration counter
            pending_muls = []      # (ready_iteration, closure)

            def push_mul(fn, delay=2):
                pending_muls.append((git[0] + delay, fn))

            def step_muls():
                while pending_muls and pending_muls[0][0] <= git[0]:
                    pending_muls.pop(0)[1]()

            def flush_muls(keep=0):
                while len(pending_muls) > keep:
                    pending_muls.pop(0)[1]()

            def normalize_pair(o_acc_pair, pair, qt):
                # Stage both unnormalized accumulators to SBUF immediately so
                # the PSUM banks free for the next unit's A@V.  v3 chain:
                # one ACT copy evacuates data+denominator together ([65,512],
                # the den rides along as row 64), DVE computes 1/den on the
                # single row ([1,512], cheap), GPSIMD broadcasts the
                # reciprocal, DVE multiplies.  vs v2 this drops the separate
                # [1,512] DVE den copy and shrinks the reciprocal from
                # [64,512] to [1,512].
                for hx, po in ((0, 0), (1, D)):
                    # o_acc rows: 0..63 = data, 64 = denominator (ones col)
                    den_t = tmp.tile([1, 512], f32, tag="dent", bufs=4,
                                     name=f"dent{pair}_{qt}_{hx}")
                    if FLAG_DEN_ACT:
                        nc.scalar.copy(out=den_t[:],
                                       in_=o_acc_pair[hx][D:D + 1])
                    else:
                        nc.vector.tensor_copy(out=den_t[:],
                                              in_=o_acc_pair[hx][D:D + 1])
                    if FLAG_MUL_PSUM:
                        # the deferred mul reads the accumulator straight
                        # from PSUM; no ACT staging copy.  The PSUM banks
                        # are held until the mul runs (one unit later),
                        # which the 2-deep ps1 rotation absorbs.
                        ou = o_acc_pair[hx]
                    else:
                        ou = tmp.tile([D, 512], f32, tag="ou", bufs=4,
                                      name=f"ou{pair}_{qt}_{hx}")
                        if FLAG_OU_SPLIT and hx == 1:
                            # stage the two heads on different engines so
                            # both PSUM accumulators free concurrently: the
                            # next unit's first A@V has a WAR on these.
                            nc.vector.tensor_copy(out=ou[:],
                                                  in_=o_acc_pair[hx][:D])
                        else:
                            nc.scalar.copy(out=ou[:], in_=o_acc_pair[hx][:D])
                    if FLAG_DIV:
                        # broadcast the raw denominator; the deferred DVE op
                        # is a single tensor_tensor divide — no reciprocal.
                        bc_sb = tmp.tile([D, 512], f32, tag="bcsb", bufs=4,
                                         name=f"bcsb{pair}_{qt}_{hx}")
                        nc.gpsimd.partition_broadcast(bc_sb[:], den_t[:],
                                                      channels=D)

                        def mul(ou=ou, bc_sb=bc_sb, po=po, pair=pair, qt=qt):
                            nc.vector.tensor_tensor(
                                out=oT_sb[po:po + D, pair,
                                          qt * 512:(qt + 1) * 512],
                                in0=ou[:D], in1=bc_sb[:],
                                op=ALU.divide)
                    elif FLAG_NORM_V3:
                        # reciprocal on the single den row BEFORE the
                        # broadcast: [1,512] instead of [64,512] on DVE.
                        # custom DVE/GPSIMD ops only read partition-0 tiles.
                        rec1 = tmp.tile([1, 512], f32, tag="rec1", bufs=4,
                                        name=f"rec1{pair}_{qt}_{hx}")
                        nc.vector.reciprocal_approx_fast(out=rec1[:],
                                                         in_=den_t[:])
                        bc_sb = tmp.tile([D, 512], f32, tag="bcsb", bufs=4,
                                         name=f"bcsb{pair}_{qt}_{hx}")
                        nc.gpsimd.partition_broadcast(bc_sb[:], rec1[:],
                                                      channels=D)

                        def mul(ou=ou, bc_sb=bc_sb, po=po, pair=pair, qt=qt):
                            nc.vector.tensor_mul(
                                out=oT_sb[po:po + D, pair,
                                          qt * 512:(qt + 1) * 512],
                                in0=ou[:D], in1=bc_sb[:])
                    else:
                        bc_sb = tmp.tile([D, 512], f32, tag="bcsb", bufs=4,
                                         name=f"bcsb{pair}_{qt}_{hx}")
                        nc.gpsimd.partition_broadcast(bc_sb[:], den_t[:],
                                                      channels=D)

                        def mul(ou=ou, bc_sb=bc_sb, po=po, pair=pair, qt=qt,
                                hx=hx):
                            rec_bc = tmp.tile([D, 512], f32, tag="recbc",
                                              bufs=4,
                                              name=f"recbc{pair}_{qt}_{hx}")
                            nc.vector.reciprocal_approx_fast(out=rec_bc[:],
                                                             in_=bc_sb[:])
                            nc.vector.tensor_mul(
                                out=oT_sb[po:po + D, pair,
                                          qt * 512:(qt + 1) * 512],
                                in0=ou[:D], in1=rec_bc[:])
                    push_mul(mul)

            def proj_mt(mt):
                # partial output projection for rows [mt*128, mt*128+128)
                pp = ps_big.tile([P, 1024], f32, tag="sc", name=f"pp{mt}")
                for nh in range(2):
                    for j in range(CG // P):
                        nc.tensor.matmul(
                            pp[:, nh * 512:nh * 512 + 512],
                            oT_sb[:, j, mt * P:(mt + 1) * P],
                            wp_sb[:, j, nh * 512:(nh + 1) * 512],
                            start=(j == 0), stop=(j == CG // P - 1))
                ysb = tmp.tile([P, 1024], f16, tag="ysb", name=f"ysb{mt}")
                # NEVER the gpsimd queue: a y-DMA trigger there waits on the
                # proj chain and would wedge partition_broadcast behind it.
                if FLAG_PP_MERGE and mt < PP_SPLIT_MT:
                    # one [128,1024] ACT op (spans both PSUM banks): the
                    # ~215ns per-op ACT overhead is paid once instead of
                    # twice.  qt2/qt3 keep split halves for tail overlap.
                    nc.scalar.copy(out=ysb[:], in_=pp[:])
                    for h2 in range(2):
                        cs = slice(h2 * 512, h2 * 512 + 512)
                        eng = (nc.sync, nc.scalar)[(2 * mt + h2) % 2]
                        eng.dma_start(yp.ap()[mt * P:(mt + 1) * P, cs],
                                      ysb[:, cs])
                else:
                    # tail projections (qt2+qt3 run in unit 7 / the drain):
                    # the ACT queue is the end-of-kernel serializer (exp g7
                    # + ou copies + pp copies back-to-back), so split the
                    # PSUM evacuation halves across ACT and the by-then
                    # idle DVE, and put BOTH yp DMA triggers on the idle
                    # sync queue instead of costing ~0.6us each on ACT.
                    for h2 in range(2):
                        cs = slice(h2 * 512, h2 * 512 + 512)
                        if h2 == 1:
                            nc.vector.tensor_copy(out=ysb[:, cs],
                                                  in_=pp[:, cs])
                        else:
                            nc.scalar.copy(out=ysb[:, cs], in_=pp[:, cs])
                        eng = (nc.sync, nc.scalar)[(2 * mt + h2) % 2]
                        eng.dma_start(yp.ap()[mt * P:(mt + 1) * P, cs],
                                      ysb[:, cs])

            units = [(pair, qt) for qt in range(QT) for pair in range(HG // 2)]
            o_accs_u = {}
            pending = []        # (u, g, exs) whose A@V is not yet emitted
            # SKEW2: A@V for group g is emitted after the scores of g+2
            # (not g+1): the ex pool (6 bufs = 3 groups) exactly covers it,
            # every A@V gets a full extra group of exp headroom, and the
            # next unit's first A@V no longer races the previous unit's
            # ou staging copies on the reused ps1 banks.
            SKEW = 2 if FLAG_SKEW2 else 1

            def emit_av(u, g, exs):
                pair, qt = units[u]
                for i in range(GROUP):
                    r = g * GROUP + i
                    for hx, h in ((0, 2 * pair), (1, 2 * pair + 1)):
                        nc.tensor.matmul(
                            o_accs_u[u][hx][:VB, :],
                            v_sb[:, r, h * VB:(h + 1) * VB],
                            exs[hx][:, i * 512:i * 512 + 512],
                            start=(r == 0), stop=(r == KV_CHUNKS - 1))
                if g == NGRP - 1:
                    # muls land via push_mul/step_muls: two iterations of
                    # deferral hide the gpsimd broadcast round-trip, and
                    # the gradual flush keeps the emission ahead of the
                    # ps1-pool reuse two units later.
                    normalize_pair(o_accs_u[u], pair, qt)
                    del o_accs_u[u]
                    if pair == 1:
                        # qt2's projection is deliberately delayed to land
                        # in unit 7 / the final drain: it keeps the PE busy
                        # (and out of the low-clock p-state) while the last
                        # unit's normalize chain runs, instead of idling
                        # 4us and then running the tail proj at 1.2GHz.
                        base = 14 if qt == 2 else 6
                        for mt4 in range(4):
                            push_filler(
                                (lambda mt: lambda: proj_mt(mt))(qt * 4 + mt4),
                                delay=base + 2 * mt4)

            for u, (pair, qt) in enumerate(units):
                qs = slice(qt * 512, (qt + 1) * 512)
                o_accs_u[u] = [ps1.tile([P, 512], f32, tag="ps1",
                                        name=f"oacc{pair}_{qt}_{i}")
                               for i in range(2)]
                if u == 0:
                    for (m, nchunk), dly in weave0:
                        push_filler(
                            (lambda a, b: lambda: qk_mtile(a, [b]))(m, nchunk),
                            delay=dly)
                if u == 1:
                    for m, nchunk in pair0_rest:
                        push_filler(
                            (lambda a, b: lambda: qk_mtile(a, [b]))(m, nchunk))
                for g in range(NGRP):
                    git[0] += 1
                    scs = [ps_big.tile([P, 1024], f32, tag="sc",
                                       name=f"sc{pair}_{qt}_{g}_{i}")
                           for i in range(2)]
                    # 64x64-mode score matmuls: per 512-wide round, 4
                    # concurrent tiles = (head pair) x (kv half of chunk).
                    for i in range(GROUP):
                        r = g * GROUP + i
                        if FLAG_SCORES_TILED:
                            for hx, po in ((0, 0), (1, D)):
                                for kvh in (0, 1):
                                    nc.tensor.matmul(
                                        scs[hx][kvh * 64:kvh * 64 + 64,
                                                i * 512:i * 512 + 512],
                                        kT_sb[po:po + D, pair,
                                              r * P + kvh * 64:
                                              r * P + kvh * 64 + 64],
                                        qT_sb[po:po + D, pair, qs],
                                        start=True, stop=True,
                                        tile_position=(po, kvh * 64))
                        else:
                            for hx, po in ((0, 0), (1, D)):
                                nc.tensor.matmul(
                                    scs[hx][:, i * 512:i * 512 + 512],
                                    kT_sb[po:po + D, pair,
                                          r * P:(r + 1) * P],
                                    qT_sb[po:po + D, pair, qs],
                                    start=True, stop=True)
                    # exp: head A on ACT (table exp); head B split between
                    # DVE (Schraudolph bit-trick, cols 0:EXP_SPLIT) and ACT
                    # (exact exp on the tail cols).  The split balances the
                    # two engines' per-group load so the A@V matmuls never
                    # wait on the slower one; per-head softmax cancels the
                    # Schraudolph systematic scale error.
                    ex0 = tmp.tile([P, 1024], bf16, tag="ex", bufs=6,
                                   name=f"ex{pair}_{qt}_{g}_0")
                    nc.scalar.activation(ex0[:], scs[0][:], AF.Exp,
                                         scale=SCALE)
                    if FLAG_EXP_DVE:
                        ex1_i = tmp.tile([P, 1024], i16, tag="exb", bufs=6,
                                         name=f"ex{pair}_{qt}_{g}_1")
                        if FLAG_EXP_CHUNK:
                            # one DVE op per kv chunk: the A@V matmuls for
                            # chunk i=0 only wait on the first half, not
                            # the whole [128,1024] op.
                            for ic in range(GROUP):
                                cs = slice(ic * 512, ic * 512 + 512)
                                nc.vector.tensor_scalar(
                                    out=ex1_i[:, cs], in0=scs[1][:, cs],
                                    scalar1=EXP_A, scalar2=EXP_B,
                                    op0=ALU.mult, op1=ALU.add)
                        else:
                            nc.vector.tensor_scalar(
                                out=ex1_i[:, 0:EXP_SPLIT],
                                in0=scs[1][:, 0:EXP_SPLIT],
                                scalar1=EXP_A, scalar2=EXP_B,
                                op0=ALU.mult, op1=ALU.add)
                            if EXP_SPLIT < 1024:
                                nc.scalar.activation(
                                    ex1_i[:, EXP_SPLIT:1024].bitcast(bf16),
                                    scs[1][:, EXP_SPLIT:1024], AF.Exp,
                                    scale=SCALE)
                        ex1 = ex1_i[:].bitcast(bf16)
                    else:
                        ex1_t = tmp.tile([P, 1024], bf16, tag="exb2", bufs=6,
                                         name=f"ex{pair}_{qt}_{g}_1")
                        nc.scalar.activation(ex1_t[:], scs[1][:], AF.Exp,
                                             scale=SCALE)
                        ex1 = ex1_t[:]
                    exs = [ex0, ex1]
                    # fillers/muls AFTER the exps: their PE matmuls deepen
                    # the score->A@V skew, and their DVE/ACT side ops queue
                    # behind this group's exps instead of delaying them.
                    if u == 0:
                        v_rtile(2 * g)
                        v_rtile(2 * g + 1)
                        if FLAG_WEAVE:
                            # two pop slots per g: the x-arrival-gated qk
                            # fillers drain without head-of-line stalls
                            pop_filler()
                            pop_filler()
                    else:
                        pop_filler()
                    step_muls()
                    pending.append((u, g, exs))
                    while len(pending) > SKEW:
                        emit_av(*pending.pop(0))
            while pending:
                emit_av(*pending.pop(0))
            flush_muls(keep=0)
            while fillers:
                fillers.pop(0)[1]()

    nc.compile()
    return nc


def _host_prep(x, w_qkv, w_proj, b_proj):
    import ml_dtypes
    bf16 = ml_dtypes.bfloat16
    wqkvT = np.ascontiguousarray(w_qkv.T).astype(bf16)   # [C, 3C]
    wpT_full = np.ascontiguousarray(w_proj.T).astype(bf16)  # [C(in), C(out)]
    in_maps = []
    for c in range(NCORES):
        b, g = divmod(c, GROUPS)
        qcols = wqkvT[:, CG * g:CG * (g + 1)]
        kcols = wqkvT[:, C + CG * g:C + CG * (g + 1)]
        vcols = wqkvT[:, 2 * C + CG * g:2 * C + CG * (g + 1)]
        # column order [k_pair0 | q_pair0 | k_pair1 | q_pair1] matches the
        # kernel's matmul emission order (see qk_mtile)
        wqk = np.ascontiguousarray(np.concatenate(
            [kcols[:, :128], qcols[:, :128],
             kcols[:, 128:], qcols[:, 128:]], axis=1))
        wv = np.ascontiguousarray(vcols)
        wp = np.ascontiguousarray(wpT_full[CG * g:CG * (g + 1), :])
        xTv = np.ascontiguousarray(x[b].T).astype(bf16)
        in_maps.append({"xT": xTv, "wqkT": wqk, "wvT": wv, "wpT": wp})
    return in_maps


def run(inputs, trace=False, nc=None):
    """Build (or reuse) the program, run on 8 cores, return (y, results)."""
    global _CACHED_NC
    from concourse.bass_utils import run_bass_kernel_spmd
    if nc is None:
        if _CACHED_NC is None:
            _CACHED_NC = _build_nc()
        nc = _CACHED_NC
    in_maps = _host_prep(**inputs)
    res = run_bass_kernel_spmd(nc, in_maps, core_ids=list(range(NCORES)),
                               trace=trace)
    bias = np.asarray(inputs["b_proj"], np.float32)
    out = np.empty((B, N, C), np.float32)
    for b in range(B):
        acc = res.results[b * GROUPS]["yp"].astype(np.float32)
        for g in range(1, GROUPS):
            acc = acc + res.results[b * GROUPS + g]["yp"]
        out[b] = acc + bias
    return out, res


def kernel(x, w_qkv, w_proj, b_proj):
    out, _ = run({"x": np.asarray(x), "w_qkv": np.asarray(w_qkv),
                  "w_proj": np.asarray(w_proj), "b_proj": np.asarray(b_proj)})
    return out

